# revision 17
# baseline (speedup 1.0000x reference)
"""Single-head causal self-attention (T=8192, C=1024, fp32) on 8 Trainium2 cores.

Sharding: interleaved over sequence. Core i owns rows {i, i+8, i+16, ...} (T/8 rows)
as both queries and keys. This balances the causal-attention work exactly across
cores while keeping the program SPMD-static: every core runs identical loop trip
counts, and all per-core differences (which rows, causal offsets) enter via input
data only.

Per-core pipeline (all matmuls bf16 inputs, fp32 PSUM accumulation):
  1. K^T / V / Q^T projections, with the K^T and V AllGathers SPLIT in half and
     issued as soon as each half of the projection output is staged to DRAM:
       K n=0 -> AG(KT0), V rows 0-511 -> AG(V0), K n=1 -> AG(KT1), V rest -> AG(V1)
     The collective stream then runs KT0,V0,KT1,V1 back-to-back while the PE
     continues with the remaining projections and the first attention chunks.
  2. Flash-style attention with scores kept TRANSPOSED (keys on partitions,
     queries on the free axis), query chunks processed in INCREASING order so
     chunk 0/1 (which only need KT0+V0) start right after the first two
     collective ops land.
       S^T[j,q] = sum_c KT[c,j] * QT[c,q]         (lhsT=KT block, rhs=QT)
       rowsum[q] = sum_j P[j,q]                   (lhsT=ones column)
       Y^T[c,q]  = sum_j V[j,c] * P[j,q]          (lhsT=V block,  rhs=P)
     exp() needs no max-subtraction: scores/sqrt(C) ~ N(0,1), max ~6 over T^2/2
     samples, safely inside fp32/bf16 exp range.
     Causal mask: keep key g_k <= query g_q with g_k = r + 8*(m0+p),
     g_q = i + 8*(q0+f). Reduces to D[p,f] = i + 8f - 8p >= thr(block), where D
     is one per-core tile and thr takes only 16 distinct values -> 16 precomputed
     0/1 comparison tiles, one extra multiply on diagonal blocks only.
  3. Output projection + bias -> per-core y rows; host reassembles y[i::8].
"""
import sys

sys.path.insert(0, "/opt/trn_rl_repo")

from contextlib import ExitStack

import numpy as np
import ml_dtypes

import concourse.bacc as bacc
import concourse.mybir as mybir
import concourse.tile as tile
from concourse.bass_utils import run_bass_kernel_spmd

P = 128
NCORES = 8

F32 = mybir.dt.float32
BF16 = mybir.dt.bfloat16
F8 = mybir.dt.float8e4
DR = mybir.MatmulPerfMode.DoubleRow
AF = mybir.ActivationFunctionType
ALU = mybir.AluOpType


def build_nc(T, C, mock_cc=False, kv_bufs=8, half_diag=True, pt_bufs=7,
             QALIGN=64, qkv_psum_bufs=8, stage_bufs=8,
             rs_pair=True, yt_bufs=4, smalls_bufs=4, rs_group=4):
    """Build the SPMD program for all 8 cores."""
    NC = NCORES
    R = T // NC          # own rows per core
    DC = C // P          # contraction chunks of 128
    QC = min(2 * P, R)   # query-chunk width (free dim of attention matmuls)
    NCH = R // QC        # query chunks per core
    NW = (R + 511) // 512  # 512-wide column chunks of R for the projections
    MBK = R // P         # key m-blocks per rank
    scale = 1.0 / float(np.sqrt(C))

    nc = bacc.Bacc("TRN2", target_bir_lowering=False, debug=False, num_devices=NC)

    # ---- kernel I/O (per-core data) ----
    xT = nc.dram_tensor("xT", [C, R], BF16, kind="ExternalInput").ap()
    wqT = nc.dram_tensor("wqT", [C, C], BF16, kind="ExternalInput").ap()
    wkT = nc.dram_tensor("wkT", [C, C], BF16, kind="ExternalInput").ap()
    wvT = nc.dram_tensor("wvT", [C, C], BF16, kind="ExternalInput").ap()
    wpT = nc.dram_tensor("wpT", [C, C], BF16, kind="ExternalInput").ap()
    bqT = nc.dram_tensor("bqT", [P, DC], F32, kind="ExternalInput").ap()
    bkT = nc.dram_tensor("bkT", [P, DC], F32, kind="ExternalInput").ap()
    bv = nc.dram_tensor("bv", [1, C], F32, kind="ExternalInput").ap()
    bp = nc.dram_tensor("bp", [1, C], F32, kind="ExternalInput").ap()
    qg = nc.dram_tensor("qg", [1, QC], F32, kind="ExternalInput").ap()   # i + 8f
    pv = nc.dram_tensor("pv", [P, 1], F32, kind="ExternalInput").ap()    # 8p
    y = nc.dram_tensor("y", [R, C], F32, kind="ExternalOutput").ap()

    with tile.TileContext(nc) as tc, ExitStack() as ctx:
        const = ctx.enter_context(tc.tile_pool(name="const", bufs=1))
        wpool = ctx.enter_context(tc.tile_pool(name="weights", bufs=1))
        dram = ctx.enter_context(tc.tile_pool(name="dram", bufs=1, space="DRAM"))

        stage = ctx.enter_context(tc.tile_pool(name="stage", bufs=stage_bufs))

        # ---- weights & xT in SBUF; q/k/v weights and xT are scoped to the
        # projection phase so their SBUF is reclaimed for the attention KV pool
        qkvw_ctx = ExitStack()
        qkvw = qkvw_ctx.enter_context(tc.tile_pool(name="qkvw", bufs=1))

        def load_pdc(pool, name, src):
            t = pool.tile([P, DC, C], BF16, tag=name, name=name)
            nc.sync.dma_start(
                out=t[:], in_=src.rearrange("(dd p) c -> p dd c", p=P)
            )
            return t

        # wk + xT first — before even the tiny const loads: the K^T phase
        # (head of the serial chain into the AllGather) only needs these two.
        # Per-dd chunks so the first matmuls start after 1/8 of the bytes land.
        wk_sb = qkvw.tile([P, DC, C], BF16, tag="wk", name="wk")
        xt_sb = qkvw.tile([P, DC, R], BF16, tag="xt")
        wkT_r = wkT.rearrange("(dd p) c -> p dd c", p=P)
        xT_r = xT.rearrange("(dd p) l -> p dd l", p=P)
        for dd in range(DC):
            nc.sync.dma_start(out=wk_sb[:, dd, :], in_=wkT_r[:, dd, :])
            nc.sync.dma_start(out=xt_sb[:, dd, :], in_=xT_r[:, dd, :])

        # ---- constants / small inputs ----
        bqT_sb = const.tile([P, DC], F32, tag="bqT")
        bkT_sb = const.tile([P, DC], F32, tag="bkT")
        nc.sync.dma_start(out=bqT_sb[:], in_=bqT[:])
        nc.sync.dma_start(out=bkT_sb[:], in_=bkT[:])
        bv_row = const.tile([1, C], F32, tag="bv_row")
        bp_row = const.tile([1, C], F32, tag="bp_row")
        nc.sync.dma_start(out=bv_row[:], in_=bv[:])
        nc.sync.dma_start(out=bp_row[:], in_=bp[:])
        bv_bc = const.tile([P, C], F32, tag="bv_bc")
        bp_bc = const.tile([P, C], F32, tag="bp_bc")
        nc.gpsimd.partition_broadcast(bv_bc[:], bv_row[:])
        nc.gpsimd.partition_broadcast(bp_bc[:], bp_row[:])
        ones_sb = const.tile([P, 1], BF16, tag="ones")
        nc.vector.memset(ones_sb[:], 1.0)

        # D[p, f] = i + NC*f - NC*p  (per-core causal helper)
        qg_sb = const.tile([1, QC], F32, tag="qg")
        pv_sb = const.tile([P, 1], F32, tag="pv")
        nc.sync.dma_start(out=qg_sb[:], in_=qg[:])
        nc.sync.dma_start(out=pv_sb[:], in_=pv[:])
        qg_bc = const.tile([P, QC], F32, tag="qg_bc")
        nc.gpsimd.partition_broadcast(qg_bc[:], qg_sb[:])
        d_sb = const.tile([P, QC], F32, tag="D")
        nc.vector.tensor_scalar(
            out=d_sb[:], in0=qg_bc[:], scalar1=pv_sb[:], scalar2=None,
            op0=ALU.subtract,
        )

        # wv + wq per-dd chunks, interleaved: the split-0 V projection and the
        # hoisted first-half Q projection each start as soon as their first
        # contraction chunks land instead of waiting for whole-tensor loads
        wv_sb = qkvw.tile([P, DC, C], BF16, tag="wv", name="wv")
        wq_sb = qkvw.tile([P, DC, C], BF16, tag="wq", name="wq")
        wvT_r = wvT.rearrange("(dd p) c -> p dd c", p=P)
        wqT_r = wqT.rearrange("(dd p) c -> p dd c", p=P)
        for dd in range(DC):
            nc.sync.dma_start(out=wv_sb[:, dd, :], in_=wvT_r[:, dd, :])
            nc.sync.dma_start(out=wq_sb[:, dd, :], in_=wqT_r[:, dd, :])
        wp_sb = load_pdc(wpool, "wp", wpT)

        # ---- internal DRAM for the collectives, split for pipelining ----
        # The K^T/V AllGathers are split along the key axis with a front-
        # loaded plan (1,1,2,4,... m-blocks) and interleaved K,V,K,V,... on
        # the one collective stream, so attention's k=0 blocks land after two
        # small ops instead of two 16MB ones. A tiny dummy AllGather issued at
        # kernel start absorbs the ~50us first-collective rank barrier while
        # the PE is busy with projections.
        def split_plan(nblk):
            plan, w = [], 1
            while nblk:
                w = min(w, nblk)
                plan.append(w)
                nblk -= w
                if len(plan) >= 2:
                    w = min(2 * w, 2)
            return plan

        kplan = split_plan(MBK)          # in P-row blocks, for both KT and V
        kstart = [sum(kplan[:s]) for s in range(len(kplan))]
        NSPL = len(kplan)

        # Each split carries BOTH its K^T m-blocks and its V rows in ONE
        # buffer (one AllGather op per split, halving the per-op floors on the
        # serial collective stream). Column layout of split s (mbs = kplan[s]):
        #   [0 : mbs*DC*P)                     K^T, key-block-tiled: col
        #       (kl*DC+mc)*P + kk = KT[mc*P+p, (kstart[s]+kl)*P + kk]
        #   [mbs*DC*P + jl*C : .. + C)         V rows: v[(kstart[s]+jl)*P + p, :]
        kv_own = [dram.tile([P, kplan[s] * (DC * P + C)], BF16,
                            name=f"kv_own{s}") for s in range(NSPL)]
        kvg = [dram.tile([NC * P, kplan[s] * (DC * P + C)], BF16,
                         addr_space="Shared", name=f"kvg{s}")
               for s in range(NSPL)]
        groups = [list(range(NC))]

        def all_gather(src, dst):
            if mock_cc:
                # timeline-sim stand-in (single-core sim rejects collectives)
                nc.sync.dma_start(out=dst[0:src.shape[0], :], in_=src[:])
            else:
                nc.gpsimd.collective_compute(
                    "AllGather", ALU.bypass, replica_groups=groups,
                    ins=[src.opt()], outs=[dst.opt()],
                )

        ktstg = qkvw_ctx.enter_context(tc.tile_pool(name="ktstg", bufs=2))

        with tc.tile_pool(name="qkv_psum", bufs=qkv_psum_bufs, space="PSUM") as qkv_psum:
            qt_sb = wpool.tile([P, DC, R], BF16, tag="qt")

            def do_q_proj(n):
                w = min(512, R - n * 512)
                for mc in range(DC):
                    ps = qkv_psum.tile([P, 512], F32, tag="ps")
                    for dd in range(DC):
                        nc.tensor.matmul(
                            ps[:, :w],
                            wq_sb[:, dd, mc * P:(mc + 1) * P],
                            xt_sb[:, dd, n * 512:n * 512 + w],
                            start=(dd == 0), stop=(dd == DC - 1),
                        )
                    nc.scalar.activation(
                        qt_sb[:, mc, n * 512:n * 512 + w], ps[:, :w],
                        AF.Identity, bias=bqT_sb[:, mc:mc + 1],
                    )

            for s in range(NSPL):
                # K^T for this split's m-blocks (c_out on partitions). Each
                # (piece, mc) eviction scatters its m-blocks into a staging
                # tile; one big DMA then writes the split's DRAM buffer and
                # the collective fires right behind it.
                mbs, k0 = kplan[s], kstart[s]
                stg = ktstg.tile([P, mbs, DC * P], BF16, tag=f"ktstg{mbs}")
                for p0 in range(0, mbs * P, 512):
                    w = min(512, mbs * P - p0)
                    for mc in range(DC):
                        ps = qkv_psum.tile([P, 512], F32, tag="ps")
                        for dd in range(DC):
                            nc.tensor.matmul(
                                ps[:, :w],
                                wk_sb[:, dd, mc * P:(mc + 1) * P],
                                xt_sb[:, dd, k0 * P + p0:k0 * P + p0 + w],
                                start=(dd == 0), stop=(dd == DC - 1),
                            )
                        for kl in range(w // P):
                            nc.scalar.activation(
                                stg[:, p0 // P + kl, mc * P:(mc + 1) * P],
                                ps[:, kl * P:(kl + 1) * P], AF.Identity,
                                bias=bkT_sb[:, mc:mc + 1],
                            )
                nc.sync.dma_start(
                    out=kv_own[s][:, :mbs * DC * P],
                    in_=stg.rearrange("p a k -> p (a k)"),
                )

                # V rows for the same split, into the same AG buffer
                voff = mbs * DC * P
                for jl in range(mbs):
                    jb = k0 + jl
                    for cn in range((C + 511) // 512):
                        ps = qkv_psum.tile([P, 512], F32, tag="ps")
                        for dd in range(DC):
                            nc.tensor.matmul(
                                ps[:],
                                xt_sb[:, dd, jb * P:(jb + 1) * P],
                                wv_sb[:, dd, cn * 512:(cn + 1) * 512],
                                start=(dd == 0), stop=(dd == DC - 1),
                            )
                        st = stage.tile([P, 512], BF16, tag="st")
                        nc.vector.tensor_add(
                            out=st[:], in0=ps[:],
                            in1=bv_bc[:, cn * 512:(cn + 1) * 512],
                        )
                        nc.sync.dma_start(
                            out=kv_own[s][:, voff + jl * C + cn * 512:
                                          voff + jl * C + (cn + 1) * 512],
                            in_=st[:],
                        )
                all_gather(kv_own[s], kvg[s])
                if s == 0:
                    # first half of Q^T now: attention chunk 0/1 queries are
                    # ready as soon as the first KV splits land
                    do_q_proj(0)

            # remaining Q^T columns (first half was emitted after split 0 so
            # the first attention chunks' queries are ready early)
            for n in range(1, NW):
                do_q_proj(n)

        qkvw_ctx.close()

        # precompute the (few) distinct causal comparison tiles
        cmp_tiles = {}

        def get_cmp(thr):
            if thr not in cmp_tiles:
                t = const.tile([P, QC], BF16, tag=f"cmp{thr}", name=f"cmp{thr}")
                nc.vector.tensor_scalar(
                    out=t[:], in0=d_sb[:], scalar1=float(thr), scalar2=None,
                    op0=ALU.is_ge,
                )
                cmp_tiles[thr] = t
            return cmp_tiles[thr]

        # ---- attention: k-outer / chunk-inner so every KV block is read from
        # HBM exactly ONCE. Y and the softmax denominator accumulate in SBUF
        # (f32) via DVE adds of each (k, chunk)'s PSUM partials; PSUM then only
        # ever holds one (k, chunk) group (4 banks Y + 2 S + 1 rowsum + 1 out).
        kv = ctx.enter_context(tc.tile_pool(name="kv", bufs=kv_bufs))
        ptp = ctx.enter_context(tc.tile_pool(name="pt", bufs=pt_bufs))
        ytp = ctx.enter_context(tc.tile_pool(name="yt", bufs=yt_bufs))
        smalls = ctx.enter_context(tc.tile_pool(name="smalls", bufs=smalls_bufs))
        accp = ctx.enter_context(tc.tile_pool(name="acc", bufs=1))
        s_psum = ctx.enter_context(tc.tile_pool(name="s_psum", bufs=2, space="PSUM"))
        y_psum = ctx.enter_context(tc.tile_pool(name="y_psum", bufs=1, space="PSUM"))
        r_psum = ctx.enter_context(tc.tile_pool(name="r_psum", bufs=1, space="PSUM"))
        p_psum = ctx.enter_context(tc.tile_pool(name="p_psum", bufs=1, space="PSUM"))

        yacc = accp.tile([P, DC, R], F32, tag="yacc")
        rsacc = accp.tile([1, R], F32, tag="rsacc")
        nc.vector.memset(yacc[:], 0.0)
        nc.vector.memset(rsacc[:], 0.0)

        def kv_split(k):
            s = max(i for i in range(NSPL) if kstart[i] <= k)
            return s, k - kstart[s]

        def load_kt(k, r):
            s, kl = kv_split(k)
            t = kv.tile([P, DC * P], BF16, tag="ktb")
            nc.sync.dma_start(
                out=t[:],
                in_=kvg[s][r * P:(r + 1) * P, kl * DC * P:(kl + 1) * DC * P],
            )
            return t

        def load_v(m0, r):
            s, jl = kv_split(m0 // P)
            voff = kplan[s] * DC * P
            t = kv.tile([P, C], BF16, tag="vb")
            nc.sync.dma_start(
                out=t[:],
                in_=kvg[s][r * P:(r + 1) * P, voff + jl * C:voff + (jl + 1) * C],
            )
            return t

        def finalize_chunk(c):
            # normalize yt = Y^T * (1/rowsum), then output projection
            recip = smalls.tile([1, QC], F32, tag="recip")
            nc.vector.reciprocal(recip[:], rsacc[:, c * QC:(c + 1) * QC])
            recip_bc = smalls.tile([P, QC], F32, tag="recip_bc")
            nc.gpsimd.partition_broadcast(recip_bc[:], recip[:])
            yt_sb = ytp.tile([P, DC, QC], BF16, tag="yt")
            for cb in range(DC):
                nc.vector.tensor_mul(
                    out=yt_sb[:, cb, :],
                    in0=yacc[:, cb, c * QC:(c + 1) * QC],
                    in1=recip_bc[:],
                )
            for qm in range(QC // P):
                for n in range((C + 511) // 512):
                    pp = p_psum.tile([P, 512], F32, tag="pp")
                    for cb in range(DC):
                        nc.tensor.matmul(
                            pp[:],
                            yt_sb[:, cb, qm * P:(qm + 1) * P],
                            wp_sb[:, cb, n * 512:(n + 1) * 512],
                            start=(cb == 0), stop=(cb == DC - 1),
                        )
                    out_sb = stage.tile([P, 512], F32, tag="out")
                    nc.vector.tensor_add(
                        out=out_sb[:], in0=pp[:], in1=bp_bc[:, n * 512:(n + 1) * 512]
                    )
                    nc.sync.dma_start(
                        out=y[c * QC + qm * P:c * QC + (qm + 1) * P,
                              n * 512:(n + 1) * 512],
                        in_=out_sb[:],
                    )

        assert NC % rs_group == 0
        for k in range(MBK):
            m0 = k * P
            blocks = [(load_kt(k, r), load_v(m0, r)) for r in range(NC)]
            c_min = max(0, (m0 - QC) // QC + 1)
            # Pack this k's eligible chunks into jobs of 1 or 2 chunks. Only
            # the diagonal chunk (odd k, c == c_min) has masked-off leading
            # query columns (qlo = QC//2); it stays single so the skip
            # applies. Adjacent non-diagonal chunks pair up: their S matmuls
            # run at N=2*QC, halving the S instruction (and LDWEIGHTS) count.
            cs = list(range(c_min, NCH))
            jobs = []
            if half_diag and k % 2 == 1 and cs:
                jobs.append((cs.pop(0), 1))
            while len(cs) >= 2:
                jobs.append((cs[0], 2))
                cs = cs[2:]
            for c in cs:
                jobs.append((c, 1))
            for c0, w in jobs:
                W = w * QC
                thr0 = NC * (m0 - QC * c0)
                qlo = 0
                if half_diag and thr0 > 0:
                    # query f is fully masked iff max_i,p(i+NC*f-NC*p) < thr
                    # i.e. f < (thr-(NC-1))/NC  ->  qlo = ceil = thr//NC
                    qlo = min(QC, max(0, thr0 // NC))
                    qlo = (qlo // QALIGN) * QALIGN
                # ---- S / exp / mask / rowsum sweep over r; pt tiles kept ----
                r_ps = r_psum.tile([1, 2 * QC], F32, tag="rs")
                pts = []
                chain = None
                for r in range(NC):
                    kt_blk, v_blk = blocks[r]
                    s_ps = s_psum.tile([P, 2 * QC], F32, tag="s")
                    for cc in range(DC):
                        nc.tensor.matmul(
                            s_ps[:, qlo:W],
                            kt_blk[:, cc * P:(cc + 1) * P],
                            qt_sb[:, cc, c0 * QC + qlo:c0 * QC + W],
                            start=(cc == 0), stop=(cc == DC - 1),
                        )
                    pt = ptp.tile([P, 2 * QC], BF16, tag="pt")
                    nc.scalar.activation(pt[:, qlo:W], s_ps[:, qlo:W], AF.Exp,
                                         scale=scale)
                    for j in range(w):
                        thr = thr0 - NC * QC * j + r
                        jlo = qlo if j == 0 else 0
                        if -NC * (P - 1) < thr:
                            nc.vector.tensor_mul(
                                out=pt[:, j * QC + jlo:(j + 1) * QC],
                                in0=pt[:, j * QC + jlo:(j + 1) * QC],
                                in1=get_cmp(thr)[:, jlo:],
                            )
                    pts.append(pt)
                    grp = r % rs_group
                    if grp == 0:
                        chain = pt
                    else:
                        # tree-sum the P tiles on DVE (bf16; ~0.1-0.2% net
                        # error on the softmax denominator) and cut the PE
                        # rowsum matmuls rs_group-fold.
                        ptsum = ptp.tile([P, 2 * QC], BF16, tag="ptsum")
                        nc.vector.tensor_add(
                            out=ptsum[:, qlo:W], in0=chain[:, qlo:W],
                            in1=pt[:, qlo:W],
                        )
                        chain = ptsum
                    if grp == rs_group - 1:
                        nc.tensor.matmul(
                            r_ps[:, qlo:W], ones_sb[:], chain[:, qlo:W],
                            start=(r == rs_group - 1), stop=(r == NC - 1),
                        )
                nc.vector.tensor_add(
                    out=rsacc[:, c0 * QC + qlo:c0 * QC + W],
                    in0=r_ps[:, qlo:W],
                    in1=rsacc[:, c0 * QC + qlo:c0 * QC + W],
                )
                # ---- Y sweeps ----
                if w == 2:
                    # paired: N=2*QC Y matmuls, 4 cb chunks per half-sweep
                    # (4 full PSUM banks), each bank drains as one contiguous
                    # [P, 2*QC] add into yacc
                    for h in range(2):
                        y_ps = [y_psum.tile([P, 2 * QC], F32, tag=f"y{t}",
                                            name=f"y_ps{t}")
                                for t in range(DC // 2)]
                        for r in range(NC):
                            v_blk = blocks[r][1]
                            pt = pts[r]
                            for cb4 in range(DC // 2):
                                cb = DC // 2 * h + cb4
                                nc.tensor.matmul(
                                    y_ps[cb4][:],
                                    v_blk[:, cb * P:(cb + 1) * P],
                                    pt[:, :W],
                                    start=(r == 0), stop=(r == NC - 1),
                                )
                        for cb4 in range(DC // 2):
                            cb = DC // 2 * h + cb4
                            nc.vector.tensor_add(
                                out=yacc[:, cb, c0 * QC:c0 * QC + W],
                                in0=y_ps[cb4][:],
                                in1=yacc[:, cb, c0 * QC:c0 * QC + W],
                            )
                    for j in range(w):
                        if k == QC * (c0 + j + 1) // P - 1:
                            finalize_chunk(c0 + j)
                else:
                    cj = c0
                    y_ps = [y_psum.tile([P, 2 * QC], F32, tag=f"y{t}",
                                        name=f"y_ps{t}") for t in range(DC // 2)]
                    for r in range(NC):
                        v_blk = blocks[r][1]
                        pt = pts[r]
                        for cb in range(DC):
                            # start=True clears has_written for the WHOLE
                            # bank, so only the first write to each bank may
                            # carry it; the second window's first write lands
                            # on cleared bits and overwrites with start=False.
                            nc.tensor.matmul(
                                y_ps[cb // 2][:, (cb % 2) * QC + qlo:
                                              (cb % 2 + 1) * QC],
                                v_blk[:, cb * P:(cb + 1) * P],
                                pt[:, qlo:QC],
                                start=(r == 0 and cb % 2 == 0),
                                stop=(r == NC - 1),
                            )
                    for cb in range(DC):
                        nc.vector.tensor_add(
                            out=yacc[:, cb, cj * QC + qlo:(cj + 1) * QC],
                            in0=y_ps[cb // 2][:, (cb % 2) * QC + qlo:
                                              (cb % 2 + 1) * QC],
                            in1=yacc[:, cb, cj * QC + qlo:(cj + 1) * QC],
                        )
                    if k == QC * (cj + 1) // P - 1:
                        finalize_chunk(cj)

    nc.finalize()
    return nc


_NC_CACHE = {}


TUNED = dict(kv_bufs=11, half_diag=True, pt_bufs=12, qkv_psum_bufs=8,
             stage_bufs=6, rs_pair=True, rs_group=8, yt_bufs=4, smalls_bufs=4)


def _get_nc(T, C):
    key = (T, C)
    if key not in _NC_CACHE:
        kwargs = TUNED if T >= 2048 else {}
        _NC_CACHE[key] = build_nc(T, C, **kwargs)
    return _NC_CACHE[key]


def build_in_maps(inputs):
    x = np.asarray(inputs["x"], dtype=np.float32)
    T, C = x.shape
    NC = NCORES
    DC = C // P
    QC = min(2 * P, T // NC)
    bf = ml_dtypes.bfloat16

    def prep_w(W):
        return np.ascontiguousarray(np.asarray(W, np.float32).T).astype(bf)

    wqT, wkT = prep_w(inputs["Wq"]), prep_w(inputs["Wk"])
    wvT, wpT = prep_w(inputs["Wv"]), prep_w(inputs["Wp"])
    bqT = np.ascontiguousarray(np.asarray(inputs["bq"], np.float32).reshape(DC, P).T)
    bkT = np.ascontiguousarray(np.asarray(inputs["bk"], np.float32).reshape(DC, P).T)
    bv_r = np.asarray(inputs["bv"], np.float32).reshape(1, C)
    bp_r = np.asarray(inputs["bp"], np.float32).reshape(1, C)
    pv = (NC * np.arange(P, dtype=np.float32)).reshape(P, 1)

    in_maps = []
    for i in range(NC):
        xTi = np.ascontiguousarray(x[i::NC].T).astype(bf)
        qg = (i + NC * np.arange(QC, dtype=np.float32)).reshape(1, QC)
        in_maps.append({
            "xT": xTi, "wqT": wqT, "wkT": wkT, "wvT": wvT, "wpT": wpT,
            "bqT": bqT, "bkT": bkT, "bv": bv_r, "bp": bp_r,
            "qg": qg, "pv": pv,
        })
    return in_maps


def kernel(x, Wq, bq, Wk, bk, Wv, bv, Wp, bp, _raw=False):
    x = np.asarray(x, dtype=np.float32)
    T, C = x.shape
    NC = NCORES

    nc = _get_nc(T, C)
    in_maps = build_in_maps(dict(x=x, Wq=Wq, bq=bq, Wk=Wk, bk=bk,
                                 Wv=Wv, bv=bv, Wp=Wp, bp=bp))

    if _raw:
        return run_bass_kernel_spmd(nc, in_maps, list(range(NC)))
    try:
        results = _run_cached(nc, T, C, in_maps)
    except Exception:
        res = run_bass_kernel_spmd(nc, in_maps, list(range(NC)))
        results = res.results
    y = np.empty((T, C), np.float32)
    for i in range(NC):
        y[i::NC] = results[i]["y"]
    return y


_RUNNER_CACHE = {}


def _run_cached(nc, T, C, in_maps):
    """Repeat-call fast path: the sharded PJRT executable and the device-side
    zero output buffers are built once; later calls only transfer inputs."""
    import jax
    from jax.sharding import Mesh, PartitionSpec, NamedSharding
    from jax.experimental.shard_map import shard_map
    import concourse.bass2jax as b2j
    import concourse.mybir as mb

    key = (T, C)
    if key not in _RUNNER_CACHE:
        b2j.install_neuronx_cc_hook()
        partition_name = (nc.partition_id_tensor.name
                          if nc.partition_id_tensor else None)
        in_names, out_names, out_avals, zero_outs = [], [], [], []
        for alloc in nc.m.functions[0].allocations:
            if not isinstance(alloc, mb.MemoryLocationSet):
                continue
            name = alloc.memorylocations[0].name
            if alloc.kind == "ExternalInput":
                if name != partition_name:
                    in_names.append(name)
            elif alloc.kind == "ExternalOutput":
                shape = tuple(alloc.tensor_shape)
                dtype = mb.dt.np(alloc.dtype)
                out_names.append(name)
                out_avals.append(jax.core.ShapedArray(shape, dtype))
                zero_outs.append(np.zeros(shape, dtype))
        n_params = len(in_names)
        all_in = in_names + out_names + ([partition_name] if partition_name else [])

        def _body(*args):
            operands = list(args)
            if partition_name is not None:
                operands.append(b2j.partition_id_tensor())
            return tuple(b2j._bass_exec_p.bind(
                *operands,
                out_avals=tuple(out_avals),
                in_names=tuple(all_in),
                out_names=tuple(out_names),
                lowering_input_output_aliases=(),
                sim_require_finite=True,
                sim_require_nnan=True,
                nc=nc,
            ))

        devices = jax.devices()[:NCORES]
        mesh = Mesh(np.asarray(devices), ("core",))
        n_outs = len(out_names)
        fn = jax.jit(
            shard_map(_body, mesh=mesh,
                      in_specs=(PartitionSpec("core"),) * (n_params + n_outs),
                      out_specs=(PartitionSpec("core"),) * n_outs,
                      check_rep=False),
            keep_unused=True,
        )
        sharding = NamedSharding(mesh, PartitionSpec("core"))
        zeros_dev = [
            jax.device_put(np.zeros((NCORES * z.shape[0], *z.shape[1:]), z.dtype),
                           sharding)
            for z in zero_outs
        ]
        _RUNNER_CACHE[key] = (fn, in_names, out_names, out_avals, zeros_dev, sharding)

    fn, in_names, out_names, out_avals, zeros_dev, sharding = _RUNNER_CACHE[key]
    import jax
    concat_in = [
        jax.device_put(
            np.concatenate([np.asarray(in_maps[c][n]) for c in range(NCORES)],
                           axis=0), sharding)
        for n in in_names
    ]
    outs = fn(*concat_in, *zeros_dev)
    results = []
    for c in range(NCORES):
        results.append({
            name: np.asarray(outs[i]).reshape(NCORES, *out_avals[i].shape)[c]
            for i, name in enumerate(out_names)
        })
    return results
